# revision 6
# baseline (speedup 1.0000x reference)
"""Trainium2 Bass kernel for the ChaoticLSTM problem.

Math (per reference):
    xW = einsum('tbi,ig->tbg', x, Wi) + B          # precompute, non-recurrent
    per step t: gates = xW[t] + h @ Wh
                i,f,o = sigmoid(...), g = tanh(...)
                c = f*c + i*g ; h = o*tanh(c)
    returns (all h_t, (h_T, c_T))

Distribution: data-parallel over batch (64 -> 8 per core, 8 cores),
Wi/Wh/B replicated; the sequence dim is the serial recurrence.

Per-core layout is fully "transposed" (feature dims on SBUF partitions,
batch on the free dim) so the per-step elementwise work is tiny:
  - phase 1: xWT[t, p, m*8+b] = (x@Wi + B) for gate (m*128+p), batch b,
    via 16 m-chunk matmuls (lhsT = Wi tile, rhs = xT chunk, f32r) staged
    to a DRAM scratch tensor.
  - phase 2: the 8 per-core batch lanes are split into TWO independent
    recurrence chains of 4 (batch elements don't interact), emitted
    interleaved so one chain's sigmoid/tanh/c-update tail overlaps the
    other chain's 64 (ldweights+matmul) burst - the PE never idles and
    stays HAM-warm.  h is kept as hT bf16 [128, 4k, 4b]; Wh is bf16
    stationary tiles; gate bias folds into the precompute.

Numerics: bf16 recurrent-matmul operands, f32r precompute; host-side
emulation showed worst-case output error ~6e-4 (relative to output
absmax) vs the fp32 reference - the recurrence is contractive, so
rounding noise saturates instead of growing.
"""

import numpy as np
import ml_dtypes

T, BS, IN, H = 2048, 64, 256, 512
N_CORES = 8
BPC = BS // N_CORES            # 8 batch per core
NCH = 2                        # independent staggered chains per core
BCH = BPC // NCH               # 4 batch per chain
G = 4 * H                      # 2048 gate width
KC = H // 128                  # 4 k-chunks of the recurrent contraction
KI = IN // 128                 # 2 k-chunks of the input contraction
MC = G // 128                  # 16 gate m-chunks
TCS = 64                       # phase-1 timestep chunk
UNROLL = 8                     # steps per For_i iteration

_BUILT = {}


def _build():
    """Build + compile the 8-core SPMD bass program (cached)."""
    if "nc" in _BUILT:
        return _BUILT["nc"]

    import concourse.bass as bass
    import concourse.tile as tile
    from concourse import bacc, mybir

    f32 = mybir.dt.float32
    f32r = mybir.dt.float32r
    bf16 = mybir.dt.bfloat16
    Sig = mybir.ActivationFunctionType.Sigmoid
    Th = mybir.ActivationFunctionType.Tanh
    Ident = mybir.ActivationFunctionType.Identity

    nc = bacc.Bacc("TRN2", target_bir_lowering=False, debug=False,
                   num_devices=N_CORES)

    xT = nc.dram_tensor("xT", [IN, T, BPC], f32r, kind="ExternalInput").ap()
    wi = nc.dram_tensor("wi", [IN, G], f32r, kind="ExternalInput").ap()
    wh = nc.dram_tensor("wh", [H, G], bf16, kind="ExternalInput").ap()
    bvec = nc.dram_tensor("bvec", [128, MC], f32, kind="ExternalInput").ap()
    outT = nc.dram_tensor("outT", [128, T, KC * BPC], f32,
                          kind="ExternalOutput").ap()
    ct_o = nc.dram_tensor("ct_o", [128, NCH, KC * BCH], f32,
                          kind="ExternalOutput").ap()
    xwt = nc.dram_tensor("xwt", [T, 128, MC * BPC], f32, kind="Internal").ap()

    HB = KC * BCH        # 16: one gate-type's free width per chain

    with tile.TileContext(nc) as tc:
        with (
            tc.tile_pool(name="singles", bufs=1) as singles,
            tc.tile_pool(name="io", bufs=3) as io,
            tc.tile_pool(name="work", bufs=3) as work,
        ):
            # ---------------- phase 1: xWT = (x @ Wi + B)^T ----------------
            bias_sb = singles.tile([128, MC], f32, tag="bias")
            nc.sync.dma_start(out=bias_sb[:], in_=bvec)
            wi_sb = []
            for k in range(KI):
                w = singles.tile([128, G], f32r, tag=f"wi{k}")
                nc.sync.dma_start(out=w[:], in_=wi[k * 128:(k + 1) * 128, :])
                wi_sb.append(w)

            with tc.tile_pool(name="ps1", bufs=2, space="PSUM") as ps1:
                for tcn in range(T // TCS):
                    xt_sb = []
                    for k in range(KI):
                        xk = io.tile([128, TCS, BPC], f32r, tag=f"xt{k}")
                        nc.sync.dma_start(
                            out=xk[:],
                            in_=xT[k * 128:(k + 1) * 128,
                                   tcn * TCS:(tcn + 1) * TCS, :])
                        xt_sb.append(xk)
                    for m in range(MC):
                        p1 = ps1.tile([128, TCS * BPC], f32, tag="p1")
                        for k in range(KI):
                            nc.tensor.matmul(
                                p1[:],
                                lhsT=wi_sb[k][:, m * 128:(m + 1) * 128],
                                rhs=xt_sb[k].rearrange("p t b -> p (t b)"),
                                start=(k == 0), stop=(k == KI - 1))
                        stg = work.tile([128, TCS * BPC], f32, tag="stg")
                        nc.scalar.activation(stg[:], p1[:], Ident,
                                             bias=bias_sb[:, m:m + 1])
                        # split write issue across SP and POOL queues
                        eng = nc.sync if (m % 2 == 0) else nc.gpsimd
                        eng.dma_start(
                            out=xwt[tcn * TCS:(tcn + 1) * TCS, :,
                                    m * BPC:(m + 1) * BPC]
                            .rearrange("t p b -> p t b"),
                            in_=stg[:])

            # ---------------- phase 2: the recurrence ----------------
            wh_sb = []
            for k in range(KC):
                w = singles.tile([128, G], bf16, tag=f"wh{k}")
                nc.sync.dma_start(out=w[:], in_=wh[k * 128:(k + 1) * 128, :])
                wh_sb.append(w)

            # per-chain state: hT bf16 [128, KC, BCH], c fp32 [128, KC*BCH]
            h_bf = [singles.tile([128, KC, BCH], bf16, tag=f"h_bf{q}",
                                 name=f"h_bf{q}")
                    for q in range(NCH)]
            c_st = [singles.tile([128, KC * BCH], f32, tag=f"c_st{q}",
                                 name=f"c_st{q}")
                    for q in range(NCH)]
            for q in range(NCH):
                nc.vector.memset(h_bf[q][:], 0.0)
                nc.vector.memset(c_st[q][:], 0.0)

            def chain_cols(xw_u, q):
                """xw columns of chain q as a [128, MC, BCH] strided view."""
                return xw_u.rearrange("p (m b) -> p m b", m=MC)[
                    :, :, q * BCH:(q + 1) * BCH]

            with tc.tile_pool(name="ps2", bufs=2, space="PSUM") as ps2:
                with tc.For_i(0, T, UNROLL,
                              hint_engines=(mybir.EngineType.PE,)) as iv:
                    # one xw load + one out store per iteration
                    xw8 = io.tile([128, UNROLL, MC * BPC], f32, tag="xw8")
                    nc.sync.dma_start(
                        out=xw8[:],
                        in_=xwt[bass.ds(iv, UNROLL)]
                        .rearrange("t p b -> p t b"))
                    hstg = io.tile([128, UNROLL, NCH, KC * BCH], f32,
                                   tag="hstg")

                    for u in range(UNROLL):
                        for q in range(NCH):
                            xw_q = chain_cols(xw8[:, u, :], q)
                            psA = ps2.tile([128, 2 * HB], f32, tag=f"psA{q}")
                            psB = ps2.tile([128, 2 * HB], f32, tag=f"psB{q}")
                            for m in range(MC):
                                if m < 8:
                                    dst = psA[:, m * BCH:(m + 1) * BCH]
                                else:
                                    dst = psB[:, (m - 8) * BCH:(m - 7) * BCH]
                                for k in range(KC):
                                    nc.tensor.matmul(
                                        dst,
                                        lhsT=wh_sb[k][:, m * 128:(m + 1) * 128],
                                        rhs=h_bf[q][:, k, :],
                                        start=(k == 0), stop=(k == KC - 1))

                            # fold in xW[t] (split so g unblocks early and
                            # only o's slice sits on the critical tail)
                            nc.vector.tensor_add(psA[:], psA[:],
                                                 xw_q[:, 0:8, :])
                            nc.vector.tensor_add(psB[:, 0:HB], psB[:, 0:HB],
                                                 xw_q[:, 8:12, :])
                            nc.vector.tensor_add(psB[:, HB:2 * HB],
                                                 psB[:, HB:2 * HB],
                                                 xw_q[:, 12:16, :])

                            if_sb = work.tile([128, 2 * HB], f32,
                                              tag=f"if{q}")
                            nc.scalar.activation(if_sb[:], psA[:], Sig)
                            g_sb = work.tile([128, HB], f32, tag=f"g{q}")
                            nc.scalar.activation(g_sb[:], psB[:, 0:HB], Th)
                            o_sb = work.tile([128, HB], f32, tag=f"o{q}")
                            nc.scalar.activation(o_sb[:], psB[:, HB:2 * HB],
                                                 Sig)

                            ig = work.tile([128, HB], f32, tag=f"ig{q}")
                            nc.vector.tensor_mul(ig[:], if_sb[:, 0:HB],
                                                 g_sb[:])
                            nc.vector.tensor_mul(c_st[q][:], c_st[q][:],
                                                 if_sb[:, HB:2 * HB])
                            nc.vector.tensor_add(c_st[q][:], c_st[q][:],
                                                 ig[:])
                            th = work.tile([128, HB], f32, tag=f"th{q}")
                            nc.scalar.activation(th[:], c_st[q][:], Th)
                            # critical-path h: bf16 for the next matmul (DVE);
                            # fp32 copy for the output staging (idle GPSIMD)
                            nc.vector.tensor_mul(
                                h_bf[q].rearrange("p k b -> p (k b)"),
                                o_sb[:], th[:])
                            nc.gpsimd.tensor_mul(hstg[:, u, q, :],
                                                 o_sb[:], th[:])

                    nc.sync.dma_start(
                        out=outT[:, bass.ds(iv, UNROLL), :],
                        in_=hstg.rearrange("p t q b -> p t (q b)"))

            for q in range(NCH):
                nc.sync.dma_start(out=ct_o[:, q, :], in_=c_st[q][:])

    nc.compile()
    _BUILT["nc"] = nc
    return nc


def kernel(x, Wi, Wh, B):
    from concourse import bass_utils

    x = np.ascontiguousarray(np.asarray(x, dtype=np.float32))
    Wi = np.ascontiguousarray(np.asarray(Wi, dtype=np.float32))
    Wh_bf = np.asarray(Wh, dtype=np.float32).astype(ml_dtypes.bfloat16)
    B_ = np.asarray(B, dtype=np.float32)
    bvec = np.ascontiguousarray(B_.reshape(MC, 128).T)  # [128, MC]

    nc = _build()

    in_maps = []
    for c in range(N_CORES):
        xs = x[:, c * BPC:(c + 1) * BPC, :]          # [T, BPC, IN]
        xT_c = np.ascontiguousarray(xs.transpose(2, 0, 1))  # [IN, T, BPC]
        in_maps.append({
            "xT": xT_c,
            "wi": Wi,
            "wh": Wh_bf,
            "bvec": bvec,
        })

    res = bass_utils.run_bass_kernel_spmd(
        nc, in_maps, core_ids=list(range(N_CORES)))

    # outT free layout: (q, k, b') with q = batch-half, b' = b % BCH
    out = np.empty((T, BS, H), dtype=np.float32)
    cT = np.empty((BS, H), dtype=np.float32)
    for c in range(N_CORES):
        o = res.results[c]["outT"]                   # [128, T, NCH*KC*BCH]
        o = o.reshape(128, T, NCH, KC, BCH)
        # h_t[q*BCH + b', k*128 + p] = o[p, t, q, k, b']
        out[:, c * BPC:(c + 1) * BPC, :] = (
            o.transpose(1, 2, 4, 3, 0).reshape(T, BPC, H))
        ct = res.results[c]["ct_o"].reshape(128, NCH, KC, BCH)
        cT[c * BPC:(c + 1) * BPC, :] = (
            ct.transpose(1, 3, 2, 0).reshape(BPC, H))
    hT = out[-1].copy()
    return out, (hT, cT)


if __name__ == "__main__":
    rng = np.random.default_rng(0)
    ins = {
        "x": rng.standard_normal((T, BS, IN)).astype(np.float32),
        "Wi": (rng.random((IN, G), dtype=np.float32) - 0.5) * 0.088,
        "Wh": (rng.random((H, G), dtype=np.float32) - 0.5) * 0.088,
        "B": (rng.random(G, dtype=np.float32) - 0.5) * 0.088,
    }
    out, (hT, cT) = kernel(**ins)
    print("out", out.shape, out.dtype, "hT", hT.shape, "cT", cT.shape)


# revision 7
# speedup vs baseline: 1.2410x; 1.2410x over previous
"""Trainium2 Bass kernel for the ChaoticLSTM problem.

Math (per reference):
    xW = einsum('tbi,ig->tbg', x, Wi) + B          # precompute, non-recurrent
    per step t: gates = xW[t] + h @ Wh
                i,f,o = sigmoid(...), g = tanh(...)
                c = f*c + i*g ; h = o*tanh(c)
    returns (all h_t, (h_T, c_T))

Distribution: data-parallel over batch (64 -> 8 per core, 8 cores),
Wi/Wh/B replicated; the sequence dim is the serial recurrence.

Per-core layout is fully "transposed" (feature dims on SBUF partitions,
batch on the free dim) so the per-step elementwise work is tiny:
  - phase 1: xWT[t, p, m*8+b] = (x@Wi + B) for gate (m*128+p), batch b,
    via 16 m-chunk matmuls (lhsT = Wi tile, rhs = xT chunk, f32r) staged
    to a DRAM scratch tensor.
  - phase 2 (recurrence): h kept as hT bf16 [128, 4, 8]; per step, 64
    (ldweights+matmul) pairs (lhsT = Wh bf16 tile [128,128], rhs = hT
    k-chunk [128,8]) accumulate gatesT into PSUM; xWT[t] is added with
    DVE; ACT applies sigmoid/tanh; DVE updates c; the bf16 h for the
    next matmul is produced directly by the critical-path DVE multiply
    while the fp32 copy for the output staging goes to the idle GPSIMD.
    xW loads and h stores are batched one-DMA-per-8-steps.

Numerics: bf16 recurrent-matmul operands, f32r precompute; host-side
emulation showed worst-case output error ~6e-4 (relative to output
absmax) vs the fp32 reference - the recurrence is contractive, so
rounding noise saturates instead of growing.
"""

import numpy as np
import ml_dtypes

T, BS, IN, H = 2048, 64, 256, 512
N_CORES = 8
BPC = BS // N_CORES            # 8 batch per core
G = 4 * H                      # 2048 gate width
KC = H // 128                  # 4 k-chunks of the recurrent contraction
KI = IN // 128                 # 2 k-chunks of the input contraction
MC = G // 128                  # 16 gate m-chunks
TCS = 64                       # phase-1 timestep chunk
UNROLL = 8                     # steps per For_i iteration

_BUILT = {}


def _build():
    """Build + compile the 8-core SPMD bass program (cached)."""
    if "nc" in _BUILT:
        return _BUILT["nc"]

    import concourse.bass as bass
    import concourse.tile as tile
    from concourse import bacc, mybir

    f32 = mybir.dt.float32
    f32r = mybir.dt.float32r
    bf16 = mybir.dt.bfloat16
    Sig = mybir.ActivationFunctionType.Sigmoid
    Th = mybir.ActivationFunctionType.Tanh
    Ident = mybir.ActivationFunctionType.Identity

    nc = bacc.Bacc("TRN2", target_bir_lowering=False, debug=False,
                   num_devices=N_CORES)

    xT = nc.dram_tensor("xT", [IN, T, BPC], f32r, kind="ExternalInput").ap()
    wi = nc.dram_tensor("wi", [IN, G], f32r, kind="ExternalInput").ap()
    wh = nc.dram_tensor("wh", [H, G], bf16, kind="ExternalInput").ap()
    bvec = nc.dram_tensor("bvec", [128, MC], f32, kind="ExternalInput").ap()
    outT = nc.dram_tensor("outT", [128, T, KC * BPC], f32,
                          kind="ExternalOutput").ap()
    ct_o = nc.dram_tensor("ct_o", [128, KC * BPC], f32,
                          kind="ExternalOutput").ap()
    xwt = nc.dram_tensor("xwt", [T, 128, MC * BPC], f32, kind="Internal").ap()

    HB = KC * BPC        # 32: one gate-type's free width

    with tile.TileContext(nc) as tc:
        with (
            tc.tile_pool(name="singles", bufs=1) as singles,
            tc.tile_pool(name="io", bufs=3) as io,
            tc.tile_pool(name="work", bufs=3) as work,
        ):
            # ---------------- phase 1: xWT = (x @ Wi + B)^T ----------------
            bias_sb = singles.tile([128, MC], f32, tag="bias")
            nc.sync.dma_start(out=bias_sb[:], in_=bvec)
            wi_sb = []
            for k in range(KI):
                w = singles.tile([128, G], f32r, tag=f"wi{k}", name=f"wi{k}")
                nc.sync.dma_start(out=w[:], in_=wi[k * 128:(k + 1) * 128, :])
                wi_sb.append(w)

            with tc.tile_pool(name="ps1", bufs=2, space="PSUM") as ps1:
                for tcn in range(T // TCS):
                    xt_sb = []
                    for k in range(KI):
                        xk = io.tile([128, TCS, BPC], f32r, tag=f"xt{k}",
                                     name=f"xt{k}")
                        nc.sync.dma_start(
                            out=xk[:],
                            in_=xT[k * 128:(k + 1) * 128,
                                   tcn * TCS:(tcn + 1) * TCS, :])
                        xt_sb.append(xk)
                    for m in range(MC):
                        p1 = ps1.tile([128, TCS * BPC], f32, tag="p1")
                        for k in range(KI):
                            nc.tensor.matmul(
                                p1[:],
                                lhsT=wi_sb[k][:, m * 128:(m + 1) * 128],
                                rhs=xt_sb[k].rearrange("p t b -> p (t b)"),
                                start=(k == 0), stop=(k == KI - 1))
                        stg = work.tile([128, TCS * BPC], f32, tag="stg")
                        nc.scalar.activation(stg[:], p1[:], Ident,
                                             bias=bias_sb[:, m:m + 1])
                        # split write issue across SP and POOL queues
                        eng = nc.sync if (m % 2 == 0) else nc.gpsimd
                        eng.dma_start(
                            out=xwt[tcn * TCS:(tcn + 1) * TCS, :,
                                    m * BPC:(m + 1) * BPC]
                            .rearrange("t p b -> p t b"),
                            in_=stg[:])

            # ---------------- phase 2: the recurrence ----------------
            wh_sb = []
            for k in range(KC):
                w = singles.tile([128, G], bf16, tag=f"wh{k}", name=f"wh{k}")
                nc.sync.dma_start(out=w[:], in_=wh[k * 128:(k + 1) * 128, :])
                wh_sb.append(w)

            h_bf = singles.tile([128, KC, BPC], bf16, tag="h_bf")
            c_st = singles.tile([128, HB], f32, tag="c_st")
            nc.vector.memset(h_bf[:], 0.0)
            nc.vector.memset(c_st[:], 0.0)

            with tc.tile_pool(name="ps2", bufs=3, space="PSUM") as ps2:
                with tc.For_i(0, T, UNROLL,
                              hint_engines=(mybir.EngineType.PE,)) as iv:
                    # one xw load + one out store per iteration
                    xw8 = io.tile([128, UNROLL, MC * BPC], f32, tag="xw8")
                    nc.sync.dma_start(
                        out=xw8[:],
                        in_=xwt[bass.ds(iv, UNROLL)]
                        .rearrange("t p b -> p t b"))
                    hstg = io.tile([128, UNROLL, HB], f32, tag="hstg")

                    for u in range(UNROLL):
                        xw = xw8[:, u, :]
                        # gatesT: psA = [i | f], psB = [g | o]
                        psA = ps2.tile([128, 2 * HB], f32, tag="psA")
                        psB = ps2.tile([128, 2 * HB], f32, tag="psB")
                        for m in range(MC):
                            if m < 8:
                                dst = psA[:, m * BPC:(m + 1) * BPC]
                            else:
                                dst = psB[:, (m - 8) * BPC:(m - 7) * BPC]
                            for k in range(KC):
                                nc.tensor.matmul(
                                    dst,
                                    lhsT=wh_sb[k][:, m * 128:(m + 1) * 128],
                                    rhs=h_bf[:, k, :],
                                    start=(k == 0), stop=(k == KC - 1))

                        # fold in xW[t]; split so g unblocks early and only
                        # o's slice sits on the critical tail
                        nc.vector.tensor_add(psA[:], psA[:], xw[:, 0:2 * HB])
                        nc.vector.tensor_add(psB[:, 0:HB], psB[:, 0:HB],
                                             xw[:, 2 * HB:3 * HB])
                        nc.vector.tensor_add(psB[:, HB:2 * HB],
                                             psB[:, HB:2 * HB],
                                             xw[:, 3 * HB:4 * HB])

                        if_sb = work.tile([128, 2 * HB], f32, tag="if_sb")
                        nc.scalar.activation(if_sb[:], psA[:], Sig)
                        g_sb = work.tile([128, HB], f32, tag="g_sb")
                        nc.scalar.activation(g_sb[:], psB[:, 0:HB], Th)
                        o_sb = work.tile([128, HB], f32, tag="o_sb")
                        nc.scalar.activation(o_sb[:], psB[:, HB:2 * HB], Sig)

                        ig = work.tile([128, HB], f32, tag="ig")
                        nc.vector.tensor_mul(ig[:], if_sb[:, 0:HB], g_sb[:])
                        nc.vector.tensor_mul(c_st[:], c_st[:],
                                             if_sb[:, HB:2 * HB])
                        nc.vector.tensor_add(c_st[:], c_st[:], ig[:])
                        th = work.tile([128, HB], f32, tag="th")
                        nc.scalar.activation(th[:], c_st[:], Th)
                        # critical-path h in bf16 (DVE); fp32 copy on GPSIMD
                        nc.vector.tensor_mul(
                            h_bf.rearrange("p k b -> p (k b)"),
                            o_sb[:], th[:])
                        nc.gpsimd.tensor_mul(hstg[:, u, :], o_sb[:], th[:])

                    nc.sync.dma_start(
                        out=outT[:, bass.ds(iv, UNROLL), :],
                        in_=hstg[:])

            nc.sync.dma_start(out=ct_o, in_=c_st[:])

    nc.compile()
    _BUILT["nc"] = nc
    return nc


def kernel(x, Wi, Wh, B):
    from concourse import bass_utils

    x = np.ascontiguousarray(np.asarray(x, dtype=np.float32))
    Wi = np.ascontiguousarray(np.asarray(Wi, dtype=np.float32))
    Wh_bf = np.asarray(Wh, dtype=np.float32).astype(ml_dtypes.bfloat16)
    B_ = np.asarray(B, dtype=np.float32)
    bvec = np.ascontiguousarray(B_.reshape(MC, 128).T)  # [128, MC]

    nc = _build()

    in_maps = []
    for c in range(N_CORES):
        xs = x[:, c * BPC:(c + 1) * BPC, :]          # [T, BPC, IN]
        xT_c = np.ascontiguousarray(xs.transpose(2, 0, 1))  # [IN, T, BPC]
        in_maps.append({
            "xT": xT_c,
            "wi": Wi,
            "wh": Wh_bf,
            "bvec": bvec,
        })

    res = bass_utils.run_bass_kernel_spmd(
        nc, in_maps, core_ids=list(range(N_CORES)))

    out = np.empty((T, BS, H), dtype=np.float32)
    cT = np.empty((BS, H), dtype=np.float32)
    for c in range(N_CORES):
        o = res.results[c]["outT"]                   # [128, T, KC*BPC]
        o = o.reshape(128, T, KC, BPC)
        # h_t[b, k*128+p] = o[p, t, k, b]
        out[:, c * BPC:(c + 1) * BPC, :] = (
            o.transpose(1, 3, 2, 0).reshape(T, BPC, H))
        ct = res.results[c]["ct_o"].reshape(128, KC, BPC)
        cT[c * BPC:(c + 1) * BPC, :] = (
            ct.transpose(2, 1, 0).reshape(BPC, H))
    hT = out[-1].copy()
    return out, (hT, cT)


if __name__ == "__main__":
    rng = np.random.default_rng(0)
    ins = {
        "x": rng.standard_normal((T, BS, IN)).astype(np.float32),
        "Wi": (rng.random((IN, G), dtype=np.float32) - 0.5) * 0.088,
        "Wh": (rng.random((H, G), dtype=np.float32) - 0.5) * 0.088,
        "B": (rng.random(G, dtype=np.float32) - 0.5) * 0.088,
    }
    out, (hT, cT) = kernel(**ins)
    print("out", out.shape, out.dtype, "hT", hT.shape, "cT", cT.shape)
